# revision 21
# baseline (speedup 1.0000x reference)
"""Cross-attention kernel for TRN2, 8 NeuronCores.

Sharding: core (b, g) = batch b (4) x head-group g (2 groups of 4 heads).
Each core computes q/k/v projections for its 4 heads on its batch, full
T x (T+2) attention for those heads, and a partial output projection
(contribution of its 4 heads to out = attn @ Wo.T). Host sums the two
partials per batch and adds the constant (bo + Wo @ bv) term.

Math notes (vs reference):
  - 1/sqrt(Dh) folded into Wq/bq host-side.
  - tanh(g) folded into the advisory-token stream host-side
    (hpTs = hp * tanh(g), bkad = bk * tanh(g)).
  - softmax computed without max-subtraction (scores are O(5), exp is
    safe in fp32/bf16 range for this data distribution).
  - v-bias handled exactly on host: since rows of softmax sum to 1,
    its contribution to the output is the constant Wo @ bv.
  - all matmuls in bf16 with fp32 PSUM accumulation.
  - output returned in bf16, upcast + partial-sum + bias on host.

Perf structure (v3):
  - a dozen dummy matmuls on memset SBUF at t=0 warm the PE clock gate
    (HAM) before the first real matmul's inputs land.
  - host packs every input so each DMA moves >=2KB-contiguous lines per
    partition; wk/wq packed per-head and the first x t-slice per-chunk
    so the first k-projection group starts ~4us in. Inputs split across
    the two HWDGE queues (SP + ACT).
  - loop order: prelude (k, v projections) then t-tile-outer attention.
    q-projection of the next t-tile and o-projection of the previous
    t-tile are emitted ONE MATMUL AT A TIME between attention chunks
    (generator-based fillers) so the in-order PE queue always has a
    ready matmul while the scalar engine catches up on softmax EXPs.
  - softmax denominator fully off PE/ACT: DVE accumulates exp chunks,
    gpsimd partition_all_reduce sums across partitions (slow but on an
    otherwise-idle engine, pipelined across tiles), DVE reciprocal +
    normalize mul.
"""

import math
import numpy as np
import ml_dtypes

import concourse.bass as bass
import concourse.mybir as mybir
import concourse.tile as tile
from concourse import bacc, bass_isa, library_config
from concourse.bass_utils import run_bass_kernel_spmd

BF16 = mybir.dt.bfloat16
F32 = mybir.dt.float32
AFT = mybir.ActivationFunctionType

P = 128
B, T, DIM = 4, 2048, 1024
NH, DH = 8, 128
HPG = 4              # heads per core
GD = HPG * DH        # 512 out-dims per core
KC = DIM // P        # 8 contraction chunks of the model dim
TT = 512             # t tile for attention
NT = T // TT         # 4 t tiles
NTC = T // P         # 16 t chunks of 128 (v layout, o-proj)
SFC = T // P         # 16 full s-chunks (key chunks of 128)
NDUMMY = 12          # HAM warm-up matmuls

_CACHE = {}


def _build():
    nc = bacc.Bacc(
        "TRN2", target_bir_lowering=False, debug=False, enable_asserts=False
    )

    d = {}
    for name, shape, dt in [
        ("xtp", [P, NT, KC, TT], BF16),
        ("wqp", [P, HPG, KC, P], BF16),
        ("wkp", [P, HPG, KC, P], BF16),
        ("wvp", [P, KC, GD], BF16),
        ("wop", [P, HPG, DIM], BF16),
        ("bqv", [P, HPG], F32),
        ("bkv", [P, HPG], F32),
        ("bkad", [P, HPG], F32),
        ("hptp", [P, KC, 2], BF16),
        ("hptsp", [P, KC, 2], BF16),
    ]:
        d[name] = nc.dram_tensor(name, shape, dt, kind="ExternalInput").ap()
    out_ap = nc.dram_tensor("out", [T, DIM], BF16, kind="ExternalOutput").ap()

    with tile.TileContext(nc) as tc:
        nc.gpsimd.load_library(library_config.attn)
        with (
            tc.tile_pool(name="big", bufs=1) as big,
            tc.tile_pool(name="expp", bufs=6) as expp,
            tc.tile_pool(name="ettp", bufs=2) as ettp,
            tc.tile_pool(name="accp", bufs=3) as accp,
            tc.tile_pool(name="recp", bufs=2) as recp,
            tc.tile_pool(name="bcap", bufs=2) as bcap,
            tc.tile_pool(name="ostg", bufs=3) as ostg,
            tc.tile_pool(name="psS", bufs=3, space="PSUM") as psS,
            tc.tile_pool(name="psO", bufs=2, space="PSUM") as psO,
            tc.tile_pool(name="psP", bufs=3, space="PSUM") as psP,
        ):
            # ---- persistent SBUF residents ----
            # qt/kt/ot are PER-HEAD tiles: a single big tile would create
            # false cross-head dependencies in Tile's access tracking and
            # stall the interleaved o-projection fillers.
            xt = big.tile([P, NT, KC, TT], BF16)
            wq = big.tile([P, HPG, KC, P], BF16)
            wk = big.tile([P, HPG, KC, P], BF16)
            wv = big.tile([P, KC, GD], BF16)
            wo = big.tile([P, HPG, DIM], BF16)
            bq_s = big.tile([P, HPG], F32)
            bk_s = big.tile([P, HPG], F32)
            bkad_s = big.tile([P, HPG], F32)
            hpt = big.tile([P, KC, 2], BF16)
            hpts = big.tile([P, KC, 2], BF16)
            qt = [big.tile([P, T], BF16, name=f"qt{i}") for i in range(HPG)]
            kt = [big.tile([P, T + 2], BF16, name=f"kt{i}")
                  for i in range(HPG)]
            vsb = big.tile([P, NTC, GD], BF16)
            vad = big.tile([2, GD], BF16)
            ot = [big.tile([P, T], BF16, name=f"ot{i}") for i in range(HPG)]
            ones_s = big.tile([P, 1], BF16)
            dmy = big.tile([P, TT], BF16)

            # ---- HAM warm-up: dummy matmuls with no DMA dependencies ----
            nc.vector.memset(ones_s[:], 1.0)
            nc.vector.memset(dmy[:], 0.0)
            for i in range(NDUMMY):
                wps = psS.tile([P, TT], F32, tag="sc", name=f"warm_{i}")
                nc.tensor.matmul(wps[:], dmy[:, 0:P], dmy[:],
                                 start=True, stop=True)

            # ---- input DMAs ----
            # sync (SP) HWDGE: x t-slice 0 per-chunk, rest whole, then wv.
            # scalar (ACT) HWDGE: wk per-head, smalls, wq per-head, wo.
            for c in range(KC):
                nc.sync.dma_start(xt[:, 0, c, :], d["xtp"][:, 0, c, :])
            nc.sync.dma_start(xt[:, 1, :, :], d["xtp"][:, 1, :, :])
            nc.sync.dma_start(wv[:], d["wvp"][:])
            nc.sync.dma_start(xt[:, 2, :, :], d["xtp"][:, 2, :, :])
            for h in range(HPG):
                nc.scalar.dma_start(wk[:, h, :, :], d["wkp"][:, h, :, :])
            nc.scalar.dma_start(hpt[:], d["hptp"][:])
            nc.scalar.dma_start(hpts[:], d["hptsp"][:])
            nc.scalar.dma_start(bq_s[:], d["bqv"][:])
            nc.scalar.dma_start(bk_s[:], d["bkv"][:])
            nc.scalar.dma_start(bkad_s[:], d["bkad"][:])
            for h in range(HPG):
                nc.scalar.dma_start(wq[:, h, :, :], d["wqp"][:, h, :, :])
            nc.scalar.dma_start(xt[:, 3, :, :], d["xtp"][:, 3, :, :])
            nc.scalar.dma_start(wo[:], d["wop"][:])

            # ---- projection building blocks ----
            def qk_proj(h, tti, w, bias, dst, lbl, on_act=True):
                """Generator: one PE matmul per step, then the drain."""
                ts = slice(tti * TT, (tti + 1) * TT)
                ps = psP.tile([P, TT], F32, tag="pp", name=f"pj_{lbl}_{h}_{tti}")
                for c in range(KC):
                    nc.tensor.matmul(
                        ps[:], w[:, h, c, :], xt[:, tti, c, :],
                        start=(c == 0), stop=(c == KC - 1),
                    )
                    yield
                if on_act:
                    nc.scalar.activation(
                        dst[h][:, ts], ps[:], AFT.Identity,
                        bias=bias[:, h : h + 1],
                    )
                else:
                    nc.vector.tensor_scalar_add(
                        dst[h][:, ts], ps[:], bias[:, h : h + 1])
                yield

            def kad_proj(h):
                ps = psP.tile([P, 2], F32, tag="pp", name=f"kad_{h}")
                for c in range(KC):
                    nc.tensor.matmul(
                        ps[:], wk[:, h, c, :], hpts[:, c, :],
                        start=(c == 0), stop=(c == KC - 1),
                    )
                nc.scalar.activation(
                    kt[h][:, T : T + 2], ps[:], AFT.Identity,
                    bias=bkad_s[:, h : h + 1],
                )

            def v_proj(tci):
                tti, sub = tci // 4, tci % 4
                ps = psP.tile([P, GD], F32, tag="pp", name=f"vp_{tci}")
                for c in range(KC):
                    nc.tensor.matmul(
                        ps[:],
                        xt[:, tti, c, sub * P : (sub + 1) * P],
                        wv[:, c, :],
                        start=(c == 0),
                        stop=(c == KC - 1),
                    )
                nc.vector.tensor_copy(vsb[:, tci, :], ps[:])

            def vad_proj():
                ps = psP.tile([2, GD], F32, tag="pp", name="vad")
                for c in range(KC):
                    nc.tensor.matmul(
                        ps[:], hpt[:, c, :], wv[:, c, :],
                        start=(c == 0), stop=(c == KC - 1),
                    )
                nc.vector.tensor_copy(vad[:], ps[:])

            # ---- output projection group (one tci, one dm-half) ----
            stg_tiles = {}

            def c_group(tci, half):
                """Generator: one PE matmul per step, then drain (+DMA)."""
                if half == 0:
                    stg_tiles[tci] = ostg.tile([P, DIM], BF16, tag="og",
                                               name=f"o_{tci}")
                stg = stg_tiles[tci]
                cps = psP.tile([P, 512], F32, tag="pp",
                               name=f"op_{tci}_{half}")
                for c in range(HPG):
                    nc.tensor.matmul(
                        cps[:],
                        ot[c][:, tci * P : (tci + 1) * P],
                        wo[:, c, half * 512 : (half + 1) * 512],
                        start=(c == 0),
                        stop=(c == HPG - 1),
                    )
                    yield
                nc.vector.tensor_copy(stg[:, half * 512 : (half + 1) * 512],
                                      cps[:])
                if half == 1:
                    nc.sync.dma_start(out_ap[tci * P : (tci + 1) * P, :],
                                      stg[:])
                yield

            # filler driver: a queue of generators, advanced one step at a
            # time between attention matmuls.
            pending = []

            def fill(n):
                while n > 0 and pending:
                    try:
                        next(pending[0])
                        n -= 1
                    except StopIteration:
                        pending.pop(0)

            def drain_fillers():
                while pending:
                    fill(1000)

            # ---- attention tile ----
            def b_tile(tti, h):
                ts = slice(tti * TT, (tti + 1) * TT)
                hs = slice(h * P, (h + 1) * P)
                ops = psO.tile([P, TT], F32, tag="av", name=f"av_{tti}_{h}")
                acc = accp.tile([P, TT], BF16, tag="acc", name=f"ac_{tti}_{h}")
                # advisory-token tail scores first: ACT computes its exp
                # early; the tail AV matmul closes the psO group at the end.
                tps = psP.tile([2, TT], F32, tag="pp", name=f"tl_{tti}_{h}")
                nc.tensor.matmul(tps[:], kt[h][:, T : T + 2], qt[h][:, ts],
                                 start=True, stop=True)
                ett = ettp.tile([2, TT], BF16, tag="ett", name=f"et_{tti}_{h}")
                nc.scalar.activation(ett[:], tps[:], AFT.Exp)

                ets = {}

                def emit_score(sc):
                    sps = psS.tile([P, TT], F32, tag="sc",
                                   name=f"s_{tti}_{h}_{sc}")
                    nc.tensor.matmul(
                        sps[:], kt[h][:, sc * P : (sc + 1) * P],
                        qt[h][:, ts], start=True, stop=True,
                    )
                    et = expp.tile([P, TT], BF16, tag="et",
                                   name=f"e_{tti}_{h}_{sc}")
                    nc.scalar.activation(et[:], sps[:], AFT.Exp)
                    ets[sc] = et
                    # the last three chunks skip the DVE accumulate: their
                    # denominator contribution goes straight into the
                    # partition-sum matmul group, so the chain never waits
                    # on DVE behind the last EXPs.
                    if sc == 1:
                        nc.vector.tensor_add(acc[:], ets[0][:], et[:])
                        nc.vector.tensor_add(acc[0:2, :], acc[0:2, :], ett[:])
                    elif 1 < sc < SFC - 3:
                        nc.vector.tensor_add(acc[:], acc[:], et[:])

                # two-chunk score runahead so the in-order PE queue never
                # waits on an EXP: AV(i) issues only after score(i+2). The
                # partition-sum matmuls are interleaved right after their
                # AV matmuls (inputs ready by construction).
                smp = psP.tile([1, TT], F32, tag="pp", name=f"sm_{tti}_{h}")
                emit_score(0)
                emit_score(1)
                for sc in range(SFC):
                    if sc + 2 < SFC:
                        emit_score(sc + 2)
                    nc.tensor.matmul(
                        ops[:], vsb[:, sc, hs],
                        (ets[sc] if sc >= SFC - 3 else ets.pop(sc))[:],
                        start=(sc == 0), stop=False,
                    )
                    if sc == SFC - 3:
                        nc.tensor.matmul(smp[:], ones_s[:], acc[:],
                                         start=True, stop=False)
                    if sc >= SFC - 3:
                        nc.tensor.matmul(smp[:], ones_s[:], ets.pop(sc)[:],
                                         start=False, stop=(sc == SFC - 1))
                    if sc < 12:
                        fill(1)
                nc.tensor.matmul(ops[:], vad[:, hs], ett[:],
                                 start=False, stop=True)
                fill(2)
                rcf = recp.tile([1, TT], F32, tag="rc", name=f"rf_{tti}_{h}")
                nc.vector.reciprocal_approx_fast(rcf[:], smp[:])
                bsb = bcap.tile([P, TT], F32, tag="bc", name=f"bs_{tti}_{h}")
                nc.gpsimd.partition_broadcast(bsb[:], rcf[0:1, :])
                nc.vector.tensor_mul(ot[h][:, ts], ops[:], bsb[:])
                fill(2)

            # ---- prelude: k (+advisory) and v interleaved to match DMA
            # arrival order, then q of head 0 / t-tile 0. q of the other
            # heads for t-tile 0 flows in as filler work. ----
            for h in range(HPG):
                for _ in qk_proj(h, 0, wk, bk_s, kt, "k", on_act=True):
                    pass
            for tti in range(1, NT):
                for h in range(HPG):
                    for _ in qk_proj(h, tti, wk, bk_s, kt, "k", on_act=True):
                        pass
                for tci in range(4 * (tti - 1), 4 * tti):
                    v_proj(tci)
            for tci in range(12, NTC):
                v_proj(tci)
            for h in range(HPG):
                kad_proj(h)
            vad_proj()
            for _ in qk_proj(0, 0, wq, bq_s, qt, "q", on_act=True):
                pass
            for h in range(1, HPG):
                pending.append(qk_proj(h, 0, wq, bq_s, qt, "q", on_act=False))

            # ---- main loop: attention tiles with q-prefetch and o-proj
            # matmuls interleaved one-at-a-time as PE filler ----
            for tti in range(NT):
                for h in range(HPG):
                    if tti < NT - 1:
                        pending.append(
                            qk_proj(h, tti + 1, wq, bq_s, qt, "q",
                                    on_act=False))
                    if tti > 0:
                        base = (tti - 1) * 4
                        for j in range(2):
                            g = h * 2 + j
                            pending.append(c_group(base + g // 2, g % 2))
                    b_tile(tti, h)
                drain_fillers()
            for g in range(8):
                pending.append(c_group(3 * 4 + g // 2, g % 2))
            drain_fillers()

    nc.compile()
    return nc


def _get_nc():
    if "nc" not in _CACHE:
        _CACHE["nc"] = _build()
    return _CACHE["nc"]


def kernel(x, h, p, Wq, bq, Wk, bk, Wv, bv, Wo, bo, g, **_):
    x = np.asarray(x, np.float32)
    h = np.asarray(h, np.float32)
    p = np.asarray(p, np.float32)
    Wq = np.asarray(Wq, np.float32)
    bq = np.asarray(bq, np.float32)
    Wk = np.asarray(Wk, np.float32)
    bk = np.asarray(bk, np.float32)
    Wv = np.asarray(Wv, np.float32)
    bv = np.asarray(bv, np.float32)
    Wo = np.asarray(Wo, np.float32)
    bo = np.asarray(bo, np.float32)
    g = np.asarray(g, np.float32)

    nc = _get_nc()
    bf = ml_dtypes.bfloat16
    s = 1.0 / math.sqrt(DH)
    gt = float(np.tanh(g[0]))
    hp = np.concatenate([h, p], axis=1)  # [B, 2, DIM]

    def pack_w_headmajor(w):  # [GD, DIM] -> [P, HPG, KC, P]
        return np.ascontiguousarray(
            w.T.reshape(KC, P, HPG, P).transpose(1, 2, 0, 3)).astype(bf)

    def pack_w(w):  # [GD, DIM] -> [P, KC, GD]
        return np.ascontiguousarray(
            w.T.reshape(KC, P, GD).transpose(1, 0, 2)).astype(bf)

    per_group = []
    for gi in range(2):
        sl = slice(gi * GD, (gi + 1) * GD)
        per_group.append({
            "wqp": pack_w_headmajor(Wq[sl] * s),
            "wkp": pack_w_headmajor(Wk[sl]),
            "wvp": pack_w(Wv[sl]),
            "wop": np.ascontiguousarray(
                Wo[:, sl].T.reshape(HPG, P, DIM).transpose(1, 0, 2)
            ).astype(bf),
            "bqv": np.ascontiguousarray((bq[sl] * s).reshape(HPG, P).T,
                                        dtype=np.float32),
            "bkv": np.ascontiguousarray(bk[sl].reshape(HPG, P).T,
                                        dtype=np.float32),
            "bkad": np.ascontiguousarray((bk[sl] * gt).reshape(HPG, P).T,
                                         dtype=np.float32),
        })

    in_maps = []
    for b in range(B):
        xtp = np.ascontiguousarray(
            x[b].reshape(NT, TT, KC, P).transpose(3, 0, 2, 1)).astype(bf)
        hptp = np.ascontiguousarray(
            hp[b].T.reshape(KC, P, 2).transpose(1, 0, 2)).astype(bf)
        hptsp = np.ascontiguousarray(
            (hp[b] * gt).T.reshape(KC, P, 2).transpose(1, 0, 2)).astype(bf)
        for gi in range(2):
            m = dict(per_group[gi])
            m["xtp"] = xtp
            m["hptp"] = hptp
            m["hptsp"] = hptsp
            in_maps.append(m)

    _CACHE["last_in_maps"] = in_maps
    res = run_bass_kernel_spmd(nc, in_maps, list(range(8)))
    outs = res.results

    const = (bo + Wo @ bv).astype(np.float32)
    out = np.empty((B, T, DIM), np.float32)
    for b in range(B):
        out[b] = (outs[2 * b]["out"].astype(np.float32)
                  + outs[2 * b + 1]["out"].astype(np.float32) + const)
    return out


# revision 22
# speedup vs baseline: 1.1881x; 1.1881x over previous
"""Cross-attention kernel for TRN2, 8 NeuronCores.

Sharding: core (b, g) = batch b (4) x head-group g (2 groups of 4 heads).
Each core computes q/k/v projections for its 4 heads on its batch, full
T x (T+2) attention for those heads, and a partial output projection
(contribution of its 4 heads to out = attn @ Wo.T). Host sums the two
partials per batch and adds the constant (bo + Wo @ bv) term.

Math notes (vs reference):
  - 1/sqrt(Dh) folded into Wq/bq host-side.
  - tanh(g) folded into the advisory-token stream host-side
    (hpTs = hp * tanh(g), bkad = bk * tanh(g)).
  - softmax computed without max-subtraction (scores are O(5), exp is
    safe in fp32/bf16 range for this data distribution).
  - v-bias handled exactly on host: since rows of softmax sum to 1,
    its contribution to the output is the constant Wo @ bv.
  - all matmuls in bf16 with fp32 PSUM accumulation.
  - output returned in bf16, upcast + partial-sum + bias on host.

Perf structure (v3):
  - a dozen dummy matmuls on memset SBUF at t=0 warm the PE clock gate
    (HAM) before the first real matmul's inputs land.
  - host packs every input so each DMA moves >=2KB-contiguous lines per
    partition; wk/wq packed per-head and the first x t-slice per-chunk
    so the first k-projection group starts ~4us in. Inputs split across
    the two HWDGE queues (SP + ACT).
  - loop order: prelude (k, v projections) then t-tile-outer attention.
    q-projection of the next t-tile and o-projection of the previous
    t-tile are emitted ONE MATMUL AT A TIME between attention chunks
    (generator-based fillers) so the in-order PE queue always has a
    ready matmul while the scalar engine catches up on softmax EXPs.
  - softmax denominator fully off PE/ACT: DVE accumulates exp chunks,
    gpsimd partition_all_reduce sums across partitions (slow but on an
    otherwise-idle engine, pipelined across tiles), DVE reciprocal +
    normalize mul.
"""

import math
import numpy as np
import ml_dtypes

import concourse.bass as bass
import concourse.mybir as mybir
import concourse.tile as tile
from concourse import bacc, bass_isa, library_config
from concourse.bass_utils import run_bass_kernel_spmd

BF16 = mybir.dt.bfloat16
F32 = mybir.dt.float32
AFT = mybir.ActivationFunctionType

P = 128
B, T, DIM = 4, 2048, 1024
NH, DH = 8, 128
HPG = 4              # heads per core
GD = HPG * DH        # 512 out-dims per core
KC = DIM // P        # 8 contraction chunks of the model dim
TT = 512             # t tile for attention
NT = T // TT         # 4 t tiles
NTC = T // P         # 16 t chunks of 128 (v layout, o-proj)
SFC = T // P         # 16 full s-chunks (key chunks of 128)
NDUMMY = 12          # HAM warm-up matmuls

_CACHE = {}


def _build():
    nc = bacc.Bacc(
        "TRN2", target_bir_lowering=False, debug=False, enable_asserts=False
    )

    d = {}
    for name, shape, dt in [
        ("xtp", [P, NT, KC, TT], BF16),
        ("wqp", [P, HPG, KC, P], BF16),
        ("wkp", [P, HPG, KC, P], BF16),
        ("wvp", [P, KC, GD], BF16),
        ("wop", [P, HPG, DIM], BF16),
        ("bqv", [P, HPG], F32),
        ("bkv", [P, HPG], F32),
        ("bkad", [P, HPG], F32),
        ("hptp", [P, KC, 2], BF16),
        ("hptsp", [P, KC, 2], BF16),
    ]:
        d[name] = nc.dram_tensor(name, shape, dt, kind="ExternalInput").ap()
    out_ap = nc.dram_tensor("out", [T, DIM], BF16, kind="ExternalOutput").ap()

    with tile.TileContext(nc) as tc:
        nc.gpsimd.load_library(library_config.attn)
        with (
            tc.tile_pool(name="big", bufs=1) as big,
            tc.tile_pool(name="expp", bufs=6) as expp,
            tc.tile_pool(name="ettp", bufs=2) as ettp,
            tc.tile_pool(name="accp", bufs=3) as accp,
            tc.tile_pool(name="recp", bufs=2) as recp,
            tc.tile_pool(name="bcap", bufs=2) as bcap,
            tc.tile_pool(name="ostg", bufs=3) as ostg,
            tc.tile_pool(name="psS", bufs=3, space="PSUM") as psS,
            tc.tile_pool(name="psO", bufs=2, space="PSUM") as psO,
            tc.tile_pool(name="psP", bufs=3, space="PSUM") as psP,
        ):
            # ---- persistent SBUF residents ----
            # qt/kt/ot are PER-HEAD tiles: a single big tile would create
            # false cross-head dependencies in Tile's access tracking and
            # stall the interleaved o-projection fillers.
            xt = big.tile([P, NT, KC, TT], BF16)
            wq = big.tile([P, HPG, KC, P], BF16)
            wk = big.tile([P, HPG, KC, P], BF16)
            wv = big.tile([P, KC, GD], BF16)
            wo = big.tile([P, HPG, DIM], BF16)
            bq_s = big.tile([P, HPG], F32)
            bk_s = big.tile([P, HPG], F32)
            bkad_s = big.tile([P, HPG], F32)
            hpt = big.tile([P, KC, 2], BF16)
            hpts = big.tile([P, KC, 2], BF16)
            qt = [big.tile([P, T], BF16, name=f"qt{i}") for i in range(HPG)]
            kt = [big.tile([P, T + 2], BF16, name=f"kt{i}")
                  for i in range(HPG)]
            vsb = big.tile([P, NTC, GD], BF16)
            vad = big.tile([2, GD], BF16)
            ot = [big.tile([P, T], BF16, name=f"ot{i}") for i in range(HPG)]
            ones_s = big.tile([P, 1], BF16)
            dmy = big.tile([P, TT], BF16)

            # ---- HAM warm-up: dummy matmuls with no DMA dependencies ----
            nc.vector.memset(ones_s[:], 1.0)
            nc.vector.memset(dmy[:], 0.0)
            for i in range(NDUMMY):
                wps = psS.tile([P, TT], F32, tag="sc", name=f"warm_{i}")
                nc.tensor.matmul(wps[:], dmy[:, 0:P], dmy[:],
                                 start=True, stop=True)

            # ---- input DMAs ----
            # sync (SP) HWDGE: x t-slice 0 per-chunk, rest whole, then wv.
            # scalar (ACT) HWDGE: wk per-head, smalls, wq per-head, wo.
            for c in range(KC):
                nc.sync.dma_start(xt[:, 0, c, :], d["xtp"][:, 0, c, :])
            nc.sync.dma_start(xt[:, 1, :, :], d["xtp"][:, 1, :, :])
            nc.sync.dma_start(wv[:], d["wvp"][:])
            nc.sync.dma_start(xt[:, 2, :, :], d["xtp"][:, 2, :, :])
            for h in range(HPG):
                nc.scalar.dma_start(wk[:, h, :, :], d["wkp"][:, h, :, :])
            nc.scalar.dma_start(hpt[:], d["hptp"][:])
            nc.scalar.dma_start(hpts[:], d["hptsp"][:])
            nc.scalar.dma_start(bq_s[:], d["bqv"][:])
            nc.scalar.dma_start(bk_s[:], d["bkv"][:])
            nc.scalar.dma_start(bkad_s[:], d["bkad"][:])
            for h in range(HPG):
                nc.scalar.dma_start(wq[:, h, :, :], d["wqp"][:, h, :, :])
            nc.scalar.dma_start(xt[:, 3, :, :], d["xtp"][:, 3, :, :])
            nc.scalar.dma_start(wo[:], d["wop"][:])

            # ---- projection building blocks ----
            def qk_proj(h, tti, w, bias, dst, lbl, on_act=True):
                """Generator: one PE matmul per step, then the drain."""
                ts = slice(tti * TT, (tti + 1) * TT)
                ps = psP.tile([P, TT], F32, tag="pp", name=f"pj_{lbl}_{h}_{tti}")
                for c in range(KC):
                    nc.tensor.matmul(
                        ps[:], w[:, h, c, :], xt[:, tti, c, :],
                        start=(c == 0), stop=(c == KC - 1),
                    )
                    yield
                if on_act:
                    nc.scalar.activation(
                        dst[h][:, ts], ps[:], AFT.Identity,
                        bias=bias[:, h : h + 1],
                    )
                else:
                    nc.vector.tensor_scalar_add(
                        dst[h][:, ts], ps[:], bias[:, h : h + 1])
                yield

            def kad_proj(h):
                ps = psP.tile([P, 2], F32, tag="pp", name=f"kad_{h}")
                for c in range(KC):
                    nc.tensor.matmul(
                        ps[:], wk[:, h, c, :], hpts[:, c, :],
                        start=(c == 0), stop=(c == KC - 1),
                    )
                nc.scalar.activation(
                    kt[h][:, T : T + 2], ps[:], AFT.Identity,
                    bias=bkad_s[:, h : h + 1],
                )

            def v_proj(tci):
                tti, sub = tci // 4, tci % 4
                ps = psP.tile([P, GD], F32, tag="pp", name=f"vp_{tci}")
                for c in range(KC):
                    nc.tensor.matmul(
                        ps[:],
                        xt[:, tti, c, sub * P : (sub + 1) * P],
                        wv[:, c, :],
                        start=(c == 0),
                        stop=(c == KC - 1),
                    )
                nc.vector.tensor_copy(vsb[:, tci, :], ps[:])

            def vad_proj():
                ps = psP.tile([2, GD], F32, tag="pp", name="vad")
                for c in range(KC):
                    nc.tensor.matmul(
                        ps[:], hpt[:, c, :], wv[:, c, :],
                        start=(c == 0), stop=(c == KC - 1),
                    )
                nc.vector.tensor_copy(vad[:], ps[:])

            # ---- output projection group (one tci, one dm-half) ----
            stg_tiles = {}

            def c_group(tci, half):
                """Generator: one PE matmul per step, then drain (+DMA)."""
                if half == 0:
                    stg_tiles[tci] = ostg.tile([P, DIM], BF16, tag="og",
                                               name=f"o_{tci}")
                stg = stg_tiles[tci]
                cps = psP.tile([P, 512], F32, tag="pp",
                               name=f"op_{tci}_{half}")
                for c in range(HPG):
                    nc.tensor.matmul(
                        cps[:],
                        ot[c][:, tci * P : (tci + 1) * P],
                        wo[:, c, half * 512 : (half + 1) * 512],
                        start=(c == 0),
                        stop=(c == HPG - 1),
                    )
                    yield
                nc.vector.tensor_copy(stg[:, half * 512 : (half + 1) * 512],
                                      cps[:])
                if half == 1:
                    nc.sync.dma_start(out_ap[tci * P : (tci + 1) * P, :],
                                      stg[:])
                yield

            # filler driver: a queue of generators, advanced one step at a
            # time between attention matmuls.
            pending = []

            def fill(n):
                while n > 0 and pending:
                    try:
                        next(pending[0])
                        n -= 1
                    except StopIteration:
                        pending.pop(0)

            def drain_fillers():
                while pending:
                    fill(1000)

            # ---- attention tile ----
            def b_tile(tti, h):
                ts = slice(tti * TT, (tti + 1) * TT)
                hs = slice(h * P, (h + 1) * P)
                ops = psO.tile([P, TT], F32, tag="av", name=f"av_{tti}_{h}")
                acc = accp.tile([P, TT], BF16, tag="acc", name=f"ac_{tti}_{h}")
                # advisory-token tail scores first: ACT computes its exp
                # early; the tail AV matmul closes the psO group at the end.
                tps = psP.tile([2, TT], F32, tag="pp", name=f"tl_{tti}_{h}")
                nc.tensor.matmul(tps[:], kt[h][:, T : T + 2], qt[h][:, ts],
                                 start=True, stop=True)
                ett = ettp.tile([2, TT], BF16, tag="ett", name=f"et_{tti}_{h}")
                nc.scalar.activation(ett[:], tps[:], AFT.Exp)

                ets = {}

                def emit_score(sc):
                    sps = psS.tile([P, TT], F32, tag="sc",
                                   name=f"s_{tti}_{h}_{sc}")
                    nc.tensor.matmul(
                        sps[:], kt[h][:, sc * P : (sc + 1) * P],
                        qt[h][:, ts], start=True, stop=True,
                    )
                    et = expp.tile([P, TT], BF16, tag="et",
                                   name=f"e_{tti}_{h}_{sc}")
                    nc.scalar.activation(et[:], sps[:], AFT.Exp)
                    ets[sc] = et
                    # the last three chunks skip the DVE accumulate: their
                    # denominator contribution goes straight into the
                    # partition-sum matmul group, so the chain never waits
                    # on DVE behind the last EXPs.
                    if sc == 1:
                        nc.vector.tensor_add(acc[:], ets[0][:], et[:])
                        nc.vector.tensor_add(acc[0:2, :], acc[0:2, :], ett[:])
                    elif 1 < sc < SFC - 3:
                        nc.vector.tensor_add(acc[:], acc[:], et[:])

                # two-chunk score runahead so the in-order PE queue never
                # waits on an EXP: AV(i) issues only after score(i+2). The
                # partition-sum matmuls are interleaved right after their
                # AV matmuls (inputs ready by construction).
                smp = None
                emit_score(0)
                emit_score(1)
                for sc in range(SFC):
                    if sc + 2 < SFC:
                        emit_score(sc + 2)
                    nc.tensor.matmul(
                        ops[:], vsb[:, sc, hs],
                        (ets[sc] if sc >= SFC - 3 else ets.pop(sc))[:],
                        start=(sc == 0), stop=False,
                    )
                    if sc == SFC - 3:
                        smp = psP.tile([1, TT], F32, tag="pp",
                                       name=f"sm_{tti}_{h}")
                        nc.tensor.matmul(smp[:], ones_s[:], acc[:],
                                         start=True, stop=False)
                    if sc >= SFC - 3:
                        nc.tensor.matmul(smp[:], ones_s[:], ets.pop(sc)[:],
                                         start=False, stop=(sc == SFC - 1))
                    if sc < 12:
                        fill(1)
                nc.tensor.matmul(ops[:], vad[:, hs], ett[:],
                                 start=False, stop=True)
                fill(2)
                rcf = recp.tile([1, TT], F32, tag="rc", name=f"rf_{tti}_{h}")
                nc.vector.reciprocal_approx_fast(rcf[:], smp[:])
                bsb = bcap.tile([P, TT], F32, tag="bc", name=f"bs_{tti}_{h}")
                nc.gpsimd.partition_broadcast(bsb[:], rcf[0:1, :])
                nc.vector.tensor_mul(ot[h][:, ts], ops[:], bsb[:])
                fill(2)

            # ---- prelude: k (+advisory) and v interleaved to match DMA
            # arrival order, then q of head 0 / t-tile 0. q of the other
            # heads for t-tile 0 flows in as filler work. ----
            for h in range(HPG):
                for _ in qk_proj(h, 0, wk, bk_s, kt, "k", on_act=True):
                    pass
            for tti in range(1, NT):
                for h in range(HPG):
                    for _ in qk_proj(h, tti, wk, bk_s, kt, "k", on_act=True):
                        pass
                for tci in range(4 * (tti - 1), 4 * tti):
                    v_proj(tci)
            for tci in range(12, NTC):
                v_proj(tci)
            for h in range(HPG):
                kad_proj(h)
            vad_proj()
            for _ in qk_proj(0, 0, wq, bq_s, qt, "q", on_act=True):
                pass
            for h in range(1, HPG):
                pending.append(qk_proj(h, 0, wq, bq_s, qt, "q", on_act=False))

            # ---- main loop: attention tiles with q-prefetch and o-proj
            # matmuls interleaved one-at-a-time as PE filler ----
            for tti in range(NT):
                for h in range(HPG):
                    if tti < NT - 1:
                        pending.append(
                            qk_proj(h, tti + 1, wq, bq_s, qt, "q",
                                    on_act=False))
                    if tti > 0:
                        base = (tti - 1) * 4
                        for j in range(2):
                            g = h * 2 + j
                            pending.append(c_group(base + g // 2, g % 2))
                    b_tile(tti, h)
                drain_fillers()
            for g in range(8):
                pending.append(c_group(3 * 4 + g // 2, g % 2))
            drain_fillers()

    nc.compile()
    return nc


def _get_nc():
    if "nc" not in _CACHE:
        _CACHE["nc"] = _build()
    return _CACHE["nc"]


def kernel(x, h, p, Wq, bq, Wk, bk, Wv, bv, Wo, bo, g, **_):
    x = np.asarray(x, np.float32)
    h = np.asarray(h, np.float32)
    p = np.asarray(p, np.float32)
    Wq = np.asarray(Wq, np.float32)
    bq = np.asarray(bq, np.float32)
    Wk = np.asarray(Wk, np.float32)
    bk = np.asarray(bk, np.float32)
    Wv = np.asarray(Wv, np.float32)
    bv = np.asarray(bv, np.float32)
    Wo = np.asarray(Wo, np.float32)
    bo = np.asarray(bo, np.float32)
    g = np.asarray(g, np.float32)

    nc = _get_nc()
    bf = ml_dtypes.bfloat16
    s = 1.0 / math.sqrt(DH)
    gt = float(np.tanh(g[0]))
    hp = np.concatenate([h, p], axis=1)  # [B, 2, DIM]

    def pack_w_headmajor(w):  # [GD, DIM] -> [P, HPG, KC, P]
        return np.ascontiguousarray(
            w.T.reshape(KC, P, HPG, P).transpose(1, 2, 0, 3)).astype(bf)

    def pack_w(w):  # [GD, DIM] -> [P, KC, GD]
        return np.ascontiguousarray(
            w.T.reshape(KC, P, GD).transpose(1, 0, 2)).astype(bf)

    per_group = []
    for gi in range(2):
        sl = slice(gi * GD, (gi + 1) * GD)
        per_group.append({
            "wqp": pack_w_headmajor(Wq[sl] * s),
            "wkp": pack_w_headmajor(Wk[sl]),
            "wvp": pack_w(Wv[sl]),
            "wop": np.ascontiguousarray(
                Wo[:, sl].T.reshape(HPG, P, DIM).transpose(1, 0, 2)
            ).astype(bf),
            "bqv": np.ascontiguousarray((bq[sl] * s).reshape(HPG, P).T,
                                        dtype=np.float32),
            "bkv": np.ascontiguousarray(bk[sl].reshape(HPG, P).T,
                                        dtype=np.float32),
            "bkad": np.ascontiguousarray((bk[sl] * gt).reshape(HPG, P).T,
                                         dtype=np.float32),
        })

    in_maps = []
    for b in range(B):
        xtp = np.ascontiguousarray(
            x[b].reshape(NT, TT, KC, P).transpose(3, 0, 2, 1)).astype(bf)
        hptp = np.ascontiguousarray(
            hp[b].T.reshape(KC, P, 2).transpose(1, 0, 2)).astype(bf)
        hptsp = np.ascontiguousarray(
            (hp[b] * gt).T.reshape(KC, P, 2).transpose(1, 0, 2)).astype(bf)
        for gi in range(2):
            m = dict(per_group[gi])
            m["xtp"] = xtp
            m["hptp"] = hptp
            m["hptsp"] = hptsp
            in_maps.append(m)

    _CACHE["last_in_maps"] = in_maps
    res = run_bass_kernel_spmd(nc, in_maps, list(range(8)))
    outs = res.results

    const = (bo + Wo @ bv).astype(np.float32)
    out = np.empty((B, T, DIM), np.float32)
    for b in range(B):
        out[b] = (outs[2 * b]["out"].astype(np.float32)
                  + outs[2 * b + 1]["out"].astype(np.float32) + const)
    return out


# revision 26
# speedup vs baseline: 1.2186x; 1.0256x over previous
"""Cross-attention kernel for TRN2, 8 NeuronCores.

Sharding: core (b, g) = batch b (4) x head-group g (2 groups of 4 heads).
Each core computes q/k/v projections for its 4 heads on its batch, full
T x (T+2) attention for those heads, and a partial output projection
(contribution of its 4 heads to out = attn @ Wo.T). Host sums the two
partials per batch and adds the constant (bo + Wo @ bv) term.

Math notes (vs reference):
  - 1/sqrt(Dh) folded into Wq/bq host-side.
  - tanh(g) folded into the advisory-token stream host-side
    (hpTs = hp * tanh(g), bkad = bk * tanh(g)).
  - softmax computed without max-subtraction (scores are O(5), exp is
    safe in fp32/bf16 range for this data distribution).
  - v-bias handled exactly on host: since rows of softmax sum to 1,
    its contribution to the output is the constant Wo @ bv.
  - all matmuls in bf16 with fp32 PSUM accumulation.
  - output returned in bf16, upcast + partial-sum + bias on host.

Perf structure (v3):
  - a dozen dummy matmuls on memset SBUF at t=0 warm the PE clock gate
    (HAM) before the first real matmul's inputs land.
  - host packs every input so each DMA moves >=2KB-contiguous lines per
    partition; wk/wq packed per-head and the first x t-slice per-chunk
    so the first k-projection group starts ~4us in. Inputs split across
    the two HWDGE queues (SP + ACT).
  - loop order: prelude (k, v projections) then t-tile-outer attention.
    q-projection of the next t-tile and o-projection of the previous
    t-tile are emitted ONE MATMUL AT A TIME between attention chunks
    (generator-based fillers) so the in-order PE queue always has a
    ready matmul while the scalar engine catches up on softmax EXPs.
  - softmax denominator fully off PE/ACT: DVE accumulates exp chunks,
    gpsimd partition_all_reduce sums across partitions (slow but on an
    otherwise-idle engine, pipelined across tiles), DVE reciprocal +
    normalize mul.
"""

import math
import numpy as np
import ml_dtypes

import concourse.bass as bass
import concourse.mybir as mybir
import concourse.tile as tile
from concourse import bacc, bass_isa, library_config
from concourse.bass_utils import run_bass_kernel_spmd

BF16 = mybir.dt.bfloat16
F32 = mybir.dt.float32
AFT = mybir.ActivationFunctionType

P = 128
B, T, DIM = 4, 2048, 1024
NH, DH = 8, 128
HPG = 4              # heads per core
GD = HPG * DH        # 512 out-dims per core
KC = DIM // P        # 8 contraction chunks of the model dim
TT = 512             # t tile for attention
NT = T // TT         # 4 t tiles
NTC = T // P         # 16 t chunks of 128 (v layout, o-proj)
SFC = T // P         # 16 full s-chunks (key chunks of 128)
NDUMMY = 12          # HAM warm-up matmuls

_CACHE = {}


def _build():
    nc = bacc.Bacc(
        "TRN2", target_bir_lowering=False, debug=False, enable_asserts=False
    )

    d = {}
    for name, shape, dt in [
        ("xtp", [P, NT, KC, TT], BF16),
        ("wqp", [P, HPG, KC, P], BF16),
        ("wkp", [P, HPG, KC, P], BF16),
        ("wvp", [P, KC, GD], BF16),
        ("wop", [P, HPG, DIM], BF16),
        ("bqv", [P, HPG], F32),
        ("bkv", [P, HPG], F32),
        ("bkad", [P, HPG], F32),
        ("hptp", [P, KC, 2], BF16),
        ("hptsp", [P, KC, 2], BF16),
    ]:
        d[name] = nc.dram_tensor(name, shape, dt, kind="ExternalInput").ap()
    out_ap = nc.dram_tensor("out", [T, DIM], BF16, kind="ExternalOutput").ap()

    with tile.TileContext(nc) as tc:
        nc.gpsimd.load_library(library_config.attn)
        with (
            tc.tile_pool(name="big", bufs=1) as big,
            tc.tile_pool(name="expp", bufs=6) as expp,
            tc.tile_pool(name="ettp", bufs=2) as ettp,
            tc.tile_pool(name="accp", bufs=3) as accp,
            tc.tile_pool(name="recp", bufs=2) as recp,
            tc.tile_pool(name="bcap", bufs=2) as bcap,
            tc.tile_pool(name="ostg", bufs=3) as ostg,
            tc.tile_pool(name="psS", bufs=3, space="PSUM") as psS,
            tc.tile_pool(name="psO", bufs=2, space="PSUM") as psO,
            tc.tile_pool(name="psP", bufs=3, space="PSUM") as psP,
        ):
            # ---- persistent SBUF residents ----
            # qt/kt/ot are PER-HEAD tiles: a single big tile would create
            # false cross-head dependencies in Tile's access tracking and
            # stall the interleaved o-projection fillers.
            xt = big.tile([P, NT, KC, TT], BF16)
            wq = big.tile([P, HPG, KC, P], BF16)
            wk = big.tile([P, HPG, KC, P], BF16)
            wv = big.tile([P, KC, GD], BF16)
            wo = big.tile([P, HPG, DIM], BF16)
            bq_s = big.tile([P, HPG], F32)
            bk_s = big.tile([P, HPG], F32)
            bkad_s = big.tile([P, HPG], F32)
            hpt = big.tile([P, KC, 2], BF16)
            hpts = big.tile([P, KC, 2], BF16)
            qt = [big.tile([P, T], BF16, name=f"qt{i}") for i in range(HPG)]
            kt = [big.tile([P, T + 2], BF16, name=f"kt{i}")
                  for i in range(HPG)]
            vsb = big.tile([P, NTC, GD], BF16)
            vad = big.tile([2, GD], BF16)
            ot = [big.tile([P, T], BF16, name=f"ot{i}") for i in range(HPG)]
            ones_s = big.tile([P, 1], BF16)
            dmy = big.tile([P, TT], BF16)

            # ---- HAM warm-up: dummy matmuls with no DMA dependencies ----
            nc.vector.memset(ones_s[:], 1.0)
            nc.vector.memset(dmy[:], 0.0)
            for i in range(NDUMMY):
                wps = psS.tile([P, TT], F32, tag="sc", name=f"warm_{i}")
                nc.tensor.matmul(wps[:], dmy[:, 0:P], dmy[:],
                                 start=True, stop=True)

            # ---- input DMAs ----
            # sync (SP) HWDGE: x t-slice 0 per-chunk, rest whole, then wv.
            # scalar (ACT) HWDGE: wk per-head, smalls, wq per-head, wo.
            for c in range(KC):
                nc.sync.dma_start(xt[:, 0, c, :], d["xtp"][:, 0, c, :])
            nc.sync.dma_start(xt[:, 1, :, :], d["xtp"][:, 1, :, :])
            nc.sync.dma_start(wv[:], d["wvp"][:])
            nc.sync.dma_start(xt[:, 2, :, :], d["xtp"][:, 2, :, :])
            nc.sync.dma_start(xt[:, 3, :, :], d["xtp"][:, 3, :, :])
            for h in range(HPG):
                nc.scalar.dma_start(wk[:, h, :, :], d["wkp"][:, h, :, :])
            nc.scalar.dma_start(hpt[:], d["hptp"][:])
            nc.scalar.dma_start(hpts[:], d["hptsp"][:])
            nc.scalar.dma_start(bq_s[:], d["bqv"][:])
            nc.scalar.dma_start(bk_s[:], d["bkv"][:])
            nc.scalar.dma_start(bkad_s[:], d["bkad"][:])
            for h in range(HPG):
                nc.scalar.dma_start(wq[:, h, :, :], d["wqp"][:, h, :, :])
            nc.scalar.dma_start(wo[:], d["wop"][:])

            # ---- projection building blocks ----
            def qk_proj(h, tti, w, bias, dst, lbl, on_act=True):
                """Generator: one PE matmul per step, then the drain."""
                ts = slice(tti * TT, (tti + 1) * TT)
                ps = psP.tile([P, TT], F32, tag="pp", name=f"pj_{lbl}_{h}_{tti}")
                for c in range(KC):
                    nc.tensor.matmul(
                        ps[:], w[:, h, c, :], xt[:, tti, c, :],
                        start=(c == 0), stop=(c == KC - 1),
                    )
                    yield
                if on_act:
                    nc.scalar.activation(
                        dst[h][:, ts], ps[:], AFT.Identity,
                        bias=bias[:, h : h + 1],
                    )
                else:
                    nc.vector.tensor_scalar_add(
                        dst[h][:, ts], ps[:], bias[:, h : h + 1])
                yield

            def kad_proj(h):
                ps = psP.tile([P, 2], F32, tag="pp", name=f"kad_{h}")
                for c in range(KC):
                    nc.tensor.matmul(
                        ps[:], wk[:, h, c, :], hpts[:, c, :],
                        start=(c == 0), stop=(c == KC - 1),
                    )
                nc.scalar.activation(
                    kt[h][:, T : T + 2], ps[:], AFT.Identity,
                    bias=bkad_s[:, h : h + 1],
                )

            def v_proj(tci):
                tti, sub = tci // 4, tci % 4
                ps = psP.tile([P, GD], F32, tag="pp", name=f"vp_{tci}")
                for c in range(KC):
                    nc.tensor.matmul(
                        ps[:],
                        xt[:, tti, c, sub * P : (sub + 1) * P],
                        wv[:, c, :],
                        start=(c == 0),
                        stop=(c == KC - 1),
                    )
                nc.vector.tensor_copy(vsb[:, tci, :], ps[:])

            def vad_proj():
                ps = psP.tile([2, GD], F32, tag="pp", name="vad")
                for c in range(KC):
                    nc.tensor.matmul(
                        ps[:], hpt[:, c, :], wv[:, c, :],
                        start=(c == 0), stop=(c == KC - 1),
                    )
                nc.vector.tensor_copy(vad[:], ps[:])

            # ---- output projection group (one tci, one dm-half) ----
            stg_tiles = {}

            def c_group(tci, half):
                """Generator: one PE matmul per step, then drain (+DMA)."""
                if half == 0:
                    stg_tiles[tci] = ostg.tile([P, DIM], BF16, tag="og",
                                               name=f"o_{tci}")
                stg = stg_tiles[tci]
                cps = psP.tile([P, 512], F32, tag="pp",
                               name=f"op_{tci}_{half}")
                for c in range(HPG):
                    nc.tensor.matmul(
                        cps[:],
                        ot[c][:, tci * P : (tci + 1) * P],
                        wo[:, c, half * 512 : (half + 1) * 512],
                        start=(c == 0),
                        stop=(c == HPG - 1),
                    )
                    yield
                nc.vector.tensor_copy(stg[:, half * 512 : (half + 1) * 512],
                                      cps[:])
                if half == 1:
                    nc.sync.dma_start(out_ap[tci * P : (tci + 1) * P, :],
                                      stg[:])
                yield

            # filler driver: a queue of generators, advanced one step at a
            # time between attention matmuls.
            pending = []

            def fill(n):
                while n > 0 and pending:
                    try:
                        next(pending[0])
                        n -= 1
                    except StopIteration:
                        pending.pop(0)

            def drain_fillers():
                while pending:
                    fill(1000)

            # ---- attention tile ----
            # each tile's advisory-tail scores + exp are computed during
            # the PREVIOUS tile (emit_tail), so the tail exp never delays
            # the chunk-exp stream at tile start.
            tails = {}

            def emit_tail(tti, h):
                ts = slice(tti * TT, (tti + 1) * TT)
                tps = psP.tile([2, TT], F32, tag="pp", name=f"tl_{tti}_{h}")
                nc.tensor.matmul(tps[:], kt[h][:, T : T + 2], qt[h][:, ts],
                                 start=True, stop=True)
                ett = ettp.tile([2, TT], BF16, tag="ett", name=f"et_{tti}_{h}")
                nc.scalar.activation(ett[:], tps[:], AFT.Exp)
                tails[(tti, h)] = ett

            def b_tile(tti, h):
                ts = slice(tti * TT, (tti + 1) * TT)
                hs = slice(h * P, (h + 1) * P)
                ett = tails.pop((tti, h))
                ops = psO.tile([P, TT], F32, tag="av", name=f"av_{tti}_{h}")
                acc = accp.tile([P, TT], BF16, tag="acc", name=f"ac_{tti}_{h}")
                ets = {}

                def emit_score(sc):
                    sps = psS.tile([P, TT], F32, tag="sc",
                                   name=f"s_{tti}_{h}_{sc}")
                    nc.tensor.matmul(
                        sps[:], kt[h][:, sc * P : (sc + 1) * P],
                        qt[h][:, ts], start=True, stop=True,
                    )
                    et = expp.tile([P, TT], BF16, tag="et",
                                   name=f"e_{tti}_{h}_{sc}")
                    nc.scalar.activation(et[:], sps[:], AFT.Exp)
                    ets[sc] = et
                    # the last two chunks skip the DVE accumulate: their
                    # denominator contribution goes straight into the
                    # partition-sum matmul group, so the chain never waits
                    # on DVE behind the last EXPs.
                    if sc == 1:
                        nc.vector.tensor_add(acc[:], ets[0][:], et[:])
                        nc.vector.tensor_add(acc[0:2, :], acc[0:2, :], ett[:])
                    elif 1 < sc < SFC - 2:
                        nc.vector.tensor_add(acc[:], acc[:], et[:])

                # three-chunk score runahead + two leading fills so the
                # in-order PE queue never waits on an EXP; partition-sum
                # matmuls interleave where their inputs are already ready.
                smp = None
                nxt = tti * HPG + h + 1
                emit_score(0)
                emit_score(1)
                emit_score(2)
                fill(2)
                for sc in range(SFC):
                    if sc + 3 < SFC:
                        emit_score(sc + 3)
                    nc.tensor.matmul(
                        ops[:], vsb[:, sc, hs],
                        (ets[sc] if sc >= SFC - 2 else ets.pop(sc))[:],
                        start=(sc == 0), stop=False,
                    )
                    if sc == 8 and nxt < NT * HPG:
                        emit_tail(nxt // HPG, nxt % HPG)
                    if sc == SFC - 2:
                        smp = psP.tile([1, TT], F32, tag="pp",
                                       name=f"sm_{tti}_{h}")
                        nc.tensor.matmul(smp[:], ones_s[:], acc[:],
                                         start=True, stop=False)
                    if sc == SFC - 1:
                        nc.tensor.matmul(smp[:], ones_s[:],
                                         ets.pop(SFC - 2)[:],
                                         start=False, stop=False)
                    if sc < SFC - 3:
                        fill(1)
                nc.tensor.matmul(ops[:], vad[:, hs], ett[:],
                                 start=False, stop=True)
                nc.tensor.matmul(smp[:], ones_s[:], ets.pop(SFC - 1)[:],
                                 start=False, stop=True)
                rcf = recp.tile([1, TT], F32, tag="rc", name=f"rf_{tti}_{h}")
                nc.vector.reciprocal_approx_fast(rcf[:], smp[:])
                bsb = bcap.tile([P, TT], F32, tag="bc", name=f"bs_{tti}_{h}")
                nc.gpsimd.partition_broadcast(bsb[:], rcf[0:1, :])
                nc.vector.tensor_mul(ot[h][:, ts], ops[:], bsb[:])
                fill(2)

            # ---- prelude: k (+advisory) and v interleaved to match DMA
            # arrival order, then q of head 0 / t-tile 0. q of the other
            # heads for t-tile 0 flows in as filler work. ----
            for h in range(HPG):
                for _ in qk_proj(h, 0, wk, bk_s, kt, "k", on_act=True):
                    pass
            for tti in range(1, NT):
                for h in range(HPG):
                    for _ in qk_proj(h, tti, wk, bk_s, kt, "k", on_act=True):
                        pass
                for tci in range(4 * (tti - 1), 4 * tti):
                    v_proj(tci)
            for tci in range(12, NTC):
                v_proj(tci)
            for h in range(HPG):
                kad_proj(h)
            vad_proj()
            for _ in qk_proj(0, 0, wq, bq_s, qt, "q", on_act=True):
                pass
            emit_tail(0, 0)
            for h in range(1, HPG):
                pending.append(qk_proj(h, 0, wq, bq_s, qt, "q", on_act=False))

            # ---- main loop: attention tiles with q-prefetch and o-proj
            # matmuls interleaved one-at-a-time as PE filler ----
            for tti in range(NT):
                for h in range(HPG):
                    if tti < NT - 1:
                        pending.append(
                            qk_proj(h, tti + 1, wq, bq_s, qt, "q",
                                    on_act=False))
                    if tti > 0:
                        base = (tti - 1) * 4
                        for j in range(2):
                            g = h * 2 + j
                            pending.append(c_group(base + g // 2, g % 2))
                    b_tile(tti, h)
                drain_fillers()
            for g in range(8):
                pending.append(c_group(3 * 4 + g // 2, g % 2))
            drain_fillers()

    nc.compile()
    return nc


def _get_nc():
    if "nc" not in _CACHE:
        _CACHE["nc"] = _build()
    return _CACHE["nc"]


def kernel(x, h, p, Wq, bq, Wk, bk, Wv, bv, Wo, bo, g, **_):
    x = np.asarray(x, np.float32)
    h = np.asarray(h, np.float32)
    p = np.asarray(p, np.float32)
    Wq = np.asarray(Wq, np.float32)
    bq = np.asarray(bq, np.float32)
    Wk = np.asarray(Wk, np.float32)
    bk = np.asarray(bk, np.float32)
    Wv = np.asarray(Wv, np.float32)
    bv = np.asarray(bv, np.float32)
    Wo = np.asarray(Wo, np.float32)
    bo = np.asarray(bo, np.float32)
    g = np.asarray(g, np.float32)

    nc = _get_nc()
    bf = ml_dtypes.bfloat16
    s = 1.0 / math.sqrt(DH)
    gt = float(np.tanh(g[0]))
    hp = np.concatenate([h, p], axis=1)  # [B, 2, DIM]

    def pack_w_headmajor(w):  # [GD, DIM] -> [P, HPG, KC, P]
        return np.ascontiguousarray(
            w.T.reshape(KC, P, HPG, P).transpose(1, 2, 0, 3)).astype(bf)

    def pack_w(w):  # [GD, DIM] -> [P, KC, GD]
        return np.ascontiguousarray(
            w.T.reshape(KC, P, GD).transpose(1, 0, 2)).astype(bf)

    per_group = []
    for gi in range(2):
        sl = slice(gi * GD, (gi + 1) * GD)
        per_group.append({
            "wqp": pack_w_headmajor(Wq[sl] * s),
            "wkp": pack_w_headmajor(Wk[sl]),
            "wvp": pack_w(Wv[sl]),
            "wop": np.ascontiguousarray(
                Wo[:, sl].T.reshape(HPG, P, DIM).transpose(1, 0, 2)
            ).astype(bf),
            "bqv": np.ascontiguousarray((bq[sl] * s).reshape(HPG, P).T,
                                        dtype=np.float32),
            "bkv": np.ascontiguousarray(bk[sl].reshape(HPG, P).T,
                                        dtype=np.float32),
            "bkad": np.ascontiguousarray((bk[sl] * gt).reshape(HPG, P).T,
                                         dtype=np.float32),
        })

    in_maps = []
    for b in range(B):
        xtp = np.ascontiguousarray(
            x[b].reshape(NT, TT, KC, P).transpose(3, 0, 2, 1)).astype(bf)
        hptp = np.ascontiguousarray(
            hp[b].T.reshape(KC, P, 2).transpose(1, 0, 2)).astype(bf)
        hptsp = np.ascontiguousarray(
            (hp[b] * gt).T.reshape(KC, P, 2).transpose(1, 0, 2)).astype(bf)
        for gi in range(2):
            m = dict(per_group[gi])
            m["xtp"] = xtp
            m["hptp"] = hptp
            m["hptsp"] = hptsp
            in_maps.append(m)

    _CACHE["last_in_maps"] = in_maps
    res = run_bass_kernel_spmd(nc, in_maps, list(range(8)))
    outs = res.results

    const = (bo + Wo @ bv).astype(np.float32)
    out = np.empty((B, T, DIM), np.float32)
    for b in range(B):
        out[b] = (outs[2 * b]["out"].astype(np.float32)
                  + outs[2 * b + 1]["out"].astype(np.float32) + const)
    return out
